# revision 17
# baseline (speedup 1.0000x reference)
"""CapsuleNet forward kernel for 8 Trainium2 NeuronCores.

Data-parallel over batch (64 images / core); the routing b_ij batch-mean
uses two half-sized AllReduces per iteration (second half overlaps the
first collective).  u_hat is never materialized.

v2: batch-innermost layout [feature, position, batch] end to end.
  conv1  : im2col A[81, (r,c,b)] built by 18 strided DMAs with 2.5KB
           segments; 100 N=512 matmuls with contiguous rhs; ReLU+bias
           fused into contiguous PSUM->SBUF crops on alternating engines
  conv2  : per mc half, 6 PSUM accumulators (one per output row oh);
           972 accumulating N=384 matmuls whose rhs is a 3-dim AP with
           contiguous 128B inner runs (full PE stream rate)
  capsule: conv2 output bounced through DRAM once ([co,s,b] write ->
           f-major [f=128t+p, b] read); squash factors per capsule
           computed f-major via selector matmuls (sum over i = partition
           groups of 8), expanded back through a 16->128 selector matmul;
           u2Tb (b-major) derived with 72 PE transposes
  routing: s_j^T = (c-scaled W)^T @ u2 72-tile accumulation; W*c built in
           ONE broadcast-AP multiply; agreement mean via rank-64 matmul +
           gpsimd mult + DVE group-reduce + selector matmuls; AllReduce
           in two r-halves so collective 0 overlaps compute of half 1.
"""

import numpy as np
import ml_dtypes

import concourse.bacc as bacc
import concourse.bass as bass
import concourse.mybir as mybir
import concourse.tile as tile
from concourse.bass_utils import run_bass_kernel_spmd

F32 = mybir.dt.float32
BF16 = mybir.dt.bfloat16
MUL = mybir.AluOpType.mult
ADD = mybir.AluOpType.add
MAX = mybir.AluOpType.max
AXX = mybir.AxisListType.X
ACT = mybir.ActivationFunctionType

NCORES = 8
B = 512
BL = B // NCORES        # 64 images per core
R, C, O, I = 1152, 10, 16, 8
F = R * I               # 9216
CO = C * O              # 160
KT = F // 128           # 72 f-tiles
S2 = 36                 # 6x6 conv2 positions per image
NIT = 3
RH = R // 2             # collective half


def _sub(ap, off, dims):
    """Arbitrary strided view (offset in elements, dims=[[step,count],..])."""
    return bass.AP(ap.tensor, ap.offset + off, [list(d) for d in dims])


def _pp(ap):
    """Partition pitch (elements per partition row) of an SBUF AP."""
    return ap.ap[0][0]


def build_nc(for_sim=False, reps=1):
    nc = bacc.Bacc("TRN2", target_bir_lowering=False, debug=False,
                   num_devices=1 if for_sim else NCORES)
    nc._for_sim = for_sim

    xin = nc.dram_tensor("xin", [(784 + 8) * BL], BF16, kind="ExternalInput").ap()
    w1t = nc.dram_tensor("w1t", [81, 256], BF16, kind="ExternalInput").ap()
    b1 = nc.dram_tensor("b1", [128, 2], F32, kind="ExternalInput").ap()
    w2s = nc.dram_tensor("w2s", [324, 128, 128], BF16, kind="ExternalInput").ap()
    b2 = nc.dram_tensor("b2", [128, 2], F32, kind="ExternalInput").ap()
    wlb = nc.dram_tensor("wlb", [F, CO], BF16, kind="ExternalInput").ap()
    wtf = nc.dram_tensor("wtf", [CO, F], BF16, kind="ExternalInput").ap()
    sel8 = nc.dram_tensor("sel8", [128, 8], F32, kind="ExternalInput").ap()
    sel2 = nc.dram_tensor("sel2", [32, 2], F32, kind="ExternalInput").ap()
    sel16 = nc.dram_tensor("sel16", [128, 16], BF16, kind="ExternalInput").ap()
    e16 = nc.dram_tensor("e16", [16, 128], BF16, kind="ExternalInput").ap()
    eye128 = nc.dram_tensor("eye128", [128, 128], BF16, kind="ExternalInput").ap()
    eyef = nc.dram_tensor("eyef", [16, 16], F32, kind="ExternalInput").ap()
    selr = nc.dram_tensor("selr", [8, 128, 128], BF16, kind="ExternalInput").ap()
    out = nc.dram_tensor("out", [BL, CO], F32, kind="ExternalOutput").ap()

    vd1 = nc.dram_tensor("vd1", [F, BL], BF16)          # f-major bounce
    cc_in0 = nc.dram_tensor("cc_in0", [C, RH], F32)
    cc_in1 = nc.dram_tensor("cc_in1", [C, RH], F32)
    cc_out0 = nc.dram_tensor("cc_out0", [C, RH], F32,
                             addr_space="Local" if for_sim else "Shared")
    cc_out1 = nc.dram_tensor("cc_out1", [C, RH], F32,
                             addr_space="Local" if for_sim else "Shared")

    with tile.TileContext(nc, num_cores=NCORES) as tc:
        for _rep in range(reps):
            _body(tc, nc, xin, w1t, b1, w2s, b2, wlb, wtf, sel8, sel2,
                  sel16, e16, eye128, eyef, selr, out, vd1,
                  cc_in0, cc_in1, cc_out0, cc_out1)
    nc.compile()
    return nc


def _body(tc, nc, xin, w1t, b1, w2s, b2, wlb, wtf, sel8, sel2, sel16, e16,
          eye128, eyef, selr, out, vd1, cc_in0, cc_in1, cc_out0, cc_out1):
    with tc.tile_pool(name="const", bufs=1) as pc, \
         tc.tile_pool(name="upers", bufs=1) as pU:

        w1t_sb = pc.tile([81, 256], BF16, tag="w1t")
        nc.sync.dma_start(w1t_sb[:], w1t)
        b1_sb = pc.tile([128, 2], F32, tag="b1")
        nc.sync.dma_start(b1_sb[:], b1)
        b2_sb = pc.tile([128, 2], F32, tag="b2")
        nc.sync.dma_start(b2_sb[:], b2)
        sel8_sb = pc.tile([128, 8], F32, tag="sel8")
        nc.sync.dma_start(sel8_sb[:], sel8)
        sel2_sb = pc.tile([32, 2], F32, tag="sel2")
        nc.sync.dma_start(sel2_sb[:], sel2)
        sel16_sb = pc.tile([128, 16], BF16, tag="sel16")
        nc.sync.dma_start(sel16_sb[:], sel16)
        e16_sb = pc.tile([16, 128], BF16, tag="e16")
        nc.sync.dma_start(e16_sb[:], e16)
        eye128_sb = pc.tile([128, 128], BF16, tag="eye128")
        nc.sync.dma_start(eye128_sb[:], eye128)
        eyef_sb = pc.tile([16, 16], F32, tag="eyef")
        nc.sync.dma_start(eyef_sb[:], eyef)
        selr_sb = pc.tile([128, 8 * 128], BF16, tag="selr")
        nc.sync.dma_start(
            _sub(selr_sb[:], 0, [[_pp(selr_sb[:]), 128], [128, 8], [1, 128]]),
            _sub(selr, 0, [[128, 128], [128 * 128, 8], [1, 128]]))
        u2R = pU.tile([128, KT * BL], BF16, tag="u2R")   # squashed u, f-major

        # ============ Phase A/B: conv1 + conv2 + capsule formation =========
        with tc.tile_pool(name="pH", bufs=1) as pH, \
             tc.tile_pool(name="pW2", bufs=8) as pW2:

            h1 = [pH.tile([128, 20 * 20 * BL], BF16, tag=f"h1_{kc}",
                          name=f"h1_{kc}") for kc in range(2)]

            with tc.tile_pool(name="pA", bufs=1) as pA, \
                 tc.tile_pool(name="ps1", bufs=4, space="PSUM") as ps1:
                # im2col incl. 8 garbage cols per row: one fully contiguous
                # 71.7KB descriptor per tap.  A[kh*9+kw, ((r*28+c)*64+b)]
                # = xT[(r+kh)*28 + c+kw, b]; cols c>=20 are garbage (never
                # read by the matmuls below).
                A = pA.tile([81, 20 * 28 * BL], BF16, tag="A")
                pa = _pp(A[:])
                for kh in range(9):
                    dst = _sub(A[:], 9 * kh * pa,
                               [[pa, 9], [1, 20 * 28 * BL]])
                    src = _sub(xin, 28 * kh * BL,
                               [[BL, 9], [1, 20 * 28 * BL]])
                    nc.sync.dma_start(dst, src)

                flip = 0
                for mc in range(2):
                    lhsT = w1t_sb[:, mc * 128:(mc + 1) * 128]
                    for r in range(20):
                        for c0, nn in ((0, 512), (512, 512), (1024, 256)):
                            ps = ps1.tile([128, 512], F32, tag="c1ps")
                            nc.tensor.matmul(
                                ps[0:128, 0:nn], lhsT,
                                A[:, r * 28 * BL + c0:r * 28 * BL + c0 + nn],
                                start=True, stop=True)
                            dst = h1[mc][:, r * 20 * BL + c0:
                                         r * 20 * BL + c0 + nn]
                            bb = b1_sb[:, mc:mc + 1]
                            if flip % 2 == 0:
                                nc.vector.tensor_scalar(dst, ps[0:128, 0:nn],
                                                        bb, 0.0,
                                                        op0=ADD, op1=MAX)
                            else:
                                nc.scalar.activation(dst, ps[0:128, 0:nn],
                                                     ACT.Relu, bias=bb)
                            flip += 1

            # conv2 + f-major capsule squash, per mc half
            with tc.tile_pool(name="pV", bufs=2) as pV, \
                 tc.tile_pool(name="pRw", bufs=2) as pRw, \
                 tc.tile_pool(name="pSq", bufs=2) as pSq, \
                 tc.tile_pool(name="pGf", bufs=1) as pGf, \
                 tc.tile_pool(name="ps2", bufs=1, space="PSUM") as ps2, \
                 tc.tile_pool(name="psN", bufs=1, space="PSUM") as psN, \
                 tc.tile_pool(name="psE", bufs=1, space="PSUM") as psE:
                pph = _pp(h1[0][:])
                for mc in range(2):
                    pss = [ps2.tile([128, 6 * BL], F32, tag=f"c2ps{oh}",
                                    name=f"c2ps{oh}_{mc}") for oh in range(6)]
                    for kc in range(2):
                        for khw in range(81):
                            kh2, kw2 = khw // 9, khw % 9
                            wch = pW2.tile([128, 128], BF16, tag="wch")
                            nc.sync.dma_start(wch[:],
                                              w2s[mc * 162 + khw * 2 + kc])
                            lhsT = wch[:]
                            for oh in range(6):
                                rhs = _sub(h1[kc][:],
                                           ((2 * oh + kh2) * 20 + kw2) * BL,
                                           [[pph, 128], [2 * BL, 6], [1, BL]])
                                nc.tensor.matmul(
                                    pss[oh][:], lhsT, rhs,
                                    start=(kc == 0 and khw == 0),
                                    stop=(kc == 1 and khw == 80))
                    # bias add -> v[co, (s, b)] bf16
                    v = pV.tile([128, S2 * BL], BF16, tag="v")
                    for oh in range(6):
                        dst = v[:, oh * 6 * BL:(oh + 1) * 6 * BL]
                        if oh % 2 == 0:
                            nc.vector.tensor_scalar(dst, pss[oh][:],
                                                    b2_sb[:, mc:mc + 1],
                                                    None, op0=ADD)
                        else:
                            nc.scalar.add(dst, pss[oh][:],
                                          b2_sb[:, mc:mc + 1])
                    # bounce: vd1[f, b] with f = (mc*128+co)*36 + s, in two
                    # co-halves so write/read/square pipeline
                    for hf in range(2):
                        nc.sync.dma_start(
                            _sub(vd1.ap(), (mc * 128 + hf * 64) * S2 * BL,
                                 [[S2 * BL, 64], [1, S2 * BL]]),
                            v[hf * 64:(hf + 1) * 64, :])
                    # f-major read: tile t (of this mc half), partition p
                    # holds f = mc*4608 + 128*t + p; tiles 18t..18t+17 = co half
                    u2w = pRw.tile([128, 36 * BL], BF16, tag="u2w")
                    sq = pSq.tile([128, 36 * BL], BF16, tag="sq")
                    for hf in range(2):
                        nc.sync.dma_start(
                            _sub(u2w[:], hf * 18 * BL,
                                 [[_pp(u2w[:]), 128], [BL, 18], [1, BL]]),
                            _sub(vd1.ap(), (mc * 128 + hf * 64) * S2 * BL,
                                 [[BL, 128], [128 * BL, 18], [1, BL]]))
                        nc.vector.tensor_mul(
                            sq[:, hf * 18 * BL:(hf + 1) * 18 * BL],
                            u2w[:, hf * 18 * BL:(hf + 1) * 18 * BL],
                            u2w[:, hf * 18 * BL:(hf + 1) * 18 * BL])
                    for g in range(5):
                        t0, t1 = g * 8, min(g * 8 + 8, 36)
                        ncol = (t1 - t0) * BL
                        n2 = psN.tile([16, 512], F32, tag="n2")
                        for t in range(t0, t1):
                            nc.tensor.matmul(
                                n2[0:16, (t - t0) * BL:(t - t0 + 1) * BL],
                                sel16_sb[:], sq[:, t * BL:(t + 1) * BL],
                                start=True, stop=True)
                        # g = sq/((1+sq)sqrt(sq)) = sqrt(sq)/(1+sq)
                        srt = pGf.tile([16, 512], F32, tag="srt")
                        nc.scalar.sqrt(srt[0:16, 0:ncol], n2[0:16, 0:ncol])
                        den = pGf.tile([16, 512], F32, tag="den")
                        nc.vector.tensor_scalar(den[0:16, 0:ncol],
                                                n2[0:16, 0:ncol], 1.0, None,
                                                op0=ADD)
                        rc = pGf.tile([16, 512], F32, tag="rc")
                        nc.vector.reciprocal_approx_fast(rc[0:16, 0:ncol],
                                                         den[0:16, 0:ncol])
                        gf = pGf.tile([16, 512], BF16, tag="gf")
                        nc.gpsimd.tensor_tensor(gf[0:16, 0:ncol],
                                                srt[0:16, 0:ncol],
                                                rc[0:16, 0:ncol], op=MUL)
                        gx = psE.tile([128, 512], F32, tag="gx")
                        nc.tensor.matmul(gx[0:128, 0:ncol], e16_sb[:],
                                         gf[0:16, 0:ncol],
                                         start=True, stop=True)
                        nc.vector.tensor_tensor(
                            u2R[:, (mc * 36 + t0) * BL:(mc * 36 + t1) * BL],
                            u2w[:, t0 * BL:t1 * BL],
                            gx[0:128, 0:ncol], op=MUL)

        # ============ transposes + routing =================================
        with tc.tile_pool(name="pB", bufs=1) as pB, \
             tc.tile_pool(name="pBs", bufs=2) as pBs, \
             tc.tile_pool(name="psq2", bufs=1) as pq:

            u2Tb = pB.tile([BL, F], BF16, tag="u2Tb")    # squashed u, b-major
            wsb = pB.tile([128, KT * CO], BF16, tag="wsb")
            wsrc = _sub(wlb, 0, [[CO, 128], [128 * CO, KT], [1, CO]])
            wdst = _sub(wsb[:], 0, [[_pp(wsb[:]), 128], [CO, KT], [1, CO]])
            nc.sync.dma_start(wdst, wsrc)
            with tc.tile_pool(name="ptr", bufs=2, space="PSUM") as ptr:
                for t in range(KT):
                    pst = ptr.tile([BL, 128], BF16, tag="tr")
                    nc.tensor.transpose(pst[:], u2R[:, t * BL:(t + 1) * BL],
                                        eye128_sb[:])
                    if t % 2 == 0:
                        nc.vector.tensor_copy(u2Tb[:, t * 128:(t + 1) * 128],
                                              pst[:])
                    else:
                        nc.scalar.copy(u2Tb[:, t * 128:(t + 1) * 128], pst[:])

            _routing(tc, nc, pB, pBs, pq, u2R, u2Tb, wsb, wtf, out,
                     sel8_sb, sel2_sb, eyef_sb, selr_sb,
                     cc_in0, cc_in1, cc_out0, cc_out1)


def _routing(tc, nc, pB, pBs, pq, u2R, u2Tb, wsb, wtf, out,
             sel8_sb, sel2_sb, eyef_sb, selr_sb,
             cc_in0, cc_in1, cc_out0, cc_out1):
        with tc.tile_pool(name="psB", bufs=2, space="PSUM") as psB, \
             tc.tile_pool(name="psM", bufs=2, space="PSUM") as psM, \
             tc.tile_pool(name="psS", bufs=1, space="PSUM") as psS:

            wt0 = pB.tile([128, F], BF16, tag="wt0")
            nc.sync.dma_start(wt0[:], wtf[0:128])
            wt1 = pB.tile([32, F], BF16, tag="wt1")
            nc.sync.dma_start(wt1[:], wtf[128:160])
            wp = pB.tile([128, KT * CO], BF16, tag="wp")
            cE = pB.tile([128, KT * C], BF16, tag="cE")
            cTr = pB.tile([128, 9 * C], BF16, tag="cTr")
            mAll = pB.tile([8, R], F32, tag="mAll")
            mAll2 = pB.tile([2, R], F32, tag="mAll2")
            bijA = pB.tile([C, R], F32, tag="bijA")
            bijB = pB.tile([C, R], F32, tag="bijB")
            csm = pB.tile([C, R], F32, tag="csm")
            v2T = pB.tile([BL, CO], F32, tag="v2T")
            v2Tb = pB.tile([BL, CO], BF16, tag="v2Tb")
            msum = pB.tile([C, R], F32, tag="msum")

            lam = 1.0 / R
            for it in range(NIT):
                if it > 0:
                    # cTr[r%128, q*10+c] = csm[c, r]  (PE transpose, 9 blocks)
                    for q in range(9):
                        pst = psB.tile([128, C], F32, tag="ctr", name="ctr",
                                       bufs=1)
                        nc.tensor.transpose(pst[:],
                                            csm[:, q * 128:(q + 1) * 128],
                                            eyef_sb[0:C, 0:C])
                        nc.vector.tensor_copy(cTr[:, q * C:(q + 1) * C], pst[:])
                    # cE[p, t*10+c] = csm[c, 16t + p//8] via selector matmuls
                    for t1 in range(8):
                        pse = psB.tile([128, 9 * C], F32, tag="cexp",
                                       name="cexp", bufs=1)
                        nc.tensor.matmul(pse[:],
                                         selr_sb[:, t1 * 128:(t1 + 1) * 128],
                                         cTr[:], start=True, stop=True)
                        nc.vector.tensor_copy(
                            _sub(cE[:], t1 * C,
                                 [[_pp(cE[:]), 128], [8 * C, 9], [1, C]]),
                            pse[:])
                    # wp = wsb * cE broadcast over o: two halves in parallel
                    ppw = _pp(wp[:])
                    pps = _pp(wsb[:])
                    ppc = _pp(cE[:])
                    HT = 60
                    nc.vector.tensor_tensor(
                        _sub(wp[:], 0, [[ppw, 128], [CO, HT], [O, C], [1, O]]),
                        _sub(wsb[:], 0, [[pps, 128], [CO, HT], [O, C], [1, O]]),
                        _sub(cE[:], 0, [[ppc, 128], [C, HT], [1, C], [0, O]]),
                        op=MUL)
                    nc.gpsimd.tensor_tensor(
                        _sub(wp[:], HT * CO,
                             [[ppw, 128], [CO, KT - HT], [O, C], [1, O]]),
                        _sub(wsb[:], HT * CO,
                             [[pps, 128], [CO, KT - HT], [O, C], [1, O]]),
                        _sub(cE[:], HT * C,
                             [[ppc, 128], [C, KT - HT], [1, C], [0, O]]),
                        op=MUL)

                # s_j^T [b, co] over 72 accumulating K-tiles
                wcur = wsb if it == 0 else wp
                ssum = psS.tile([BL, CO], F32, tag="ssum")
                for t in range(KT):
                    nc.tensor.matmul(ssum[:], u2R[:, t * BL:(t + 1) * BL],
                                     wcur[:, t * CO:(t + 1) * CO],
                                     start=(t == 0), stop=(t == KT - 1))

                # v2 = squash(s) over o-groups of 16 (iter0 folds the 1/R scale)
                ssb = pq.tile([BL, CO], F32, tag="ssb")
                nc.vector.tensor_copy(ssb[:], ssum[:])
                svr = pq.tile([BL, CO], F32, tag="svr")
                nc.vector.tensor_mul(svr[:], ssb[:], ssb[:])
                sqv = pq.tile([BL, C], F32, tag="sqv")
                nc.vector.tensor_reduce(sqv[:],
                                        svr[:].rearrange("p (c o) -> p c o", o=O),
                                        axis=AXX, op=ADD)
                if it == 0:
                    nc.vector.tensor_scalar(sqv[:], sqv[:], lam * lam, None,
                                            op0=MUL)
                srtv = pq.tile([BL, C], F32, tag="srtv")
                nc.scalar.sqrt(srtv[:], sqv[:])
                dv2 = pq.tile([BL, C], F32, tag="dv2")
                nc.vector.scalar_tensor_tensor(dv2[:], sqv[:], 1.0, srtv[:],
                                               op0=ADD, op1=MUL)
                rcv = pq.tile([BL, C], F32, tag="rcv")
                nc.vector.reciprocal(rcv[:], dv2[:])
                gv = pq.tile([BL, C], F32, tag="gv")
                nc.vector.tensor_mul(gv[:], sqv[:], rcv[:])
                if it == 0:
                    nc.vector.tensor_scalar(gv[:], gv[:], lam, None, op0=MUL)
                # v2T[b, (c,o)] = ssb * gv broadcast over o: one pass
                ppv = _pp(v2T[:])
                pps2 = _pp(ssb[:])
                ppg = _pp(gv[:])
                nc.vector.tensor_tensor(
                    _sub(v2T[:], 0, [[ppv, BL], [O, C], [1, O]]),
                    _sub(ssb[:], 0, [[pps2, BL], [O, C], [1, O]]),
                    _sub(gv[:], 0, [[ppg, BL], [1, C], [0, O]]),
                    op=MUL)

                if it == NIT - 1:
                    nc.sync.dma_start(out, v2T[:])
                    break

                nc.vector.tensor_copy(v2Tb[:], v2T[:])
                # m[c, r] = sum_{o,i} Wt[(c,o),(r,i)] * (v2^T @ u2)[(c,o),(r,i)]
                # r-blocks of 64 outer so the collective can go out in halves
                for rb in range(18):
                    f0 = rb * 512
                    for mc2 in range(2):
                        npart = 128 if mc2 == 0 else 32
                        ncls = 8 if mc2 == 0 else 2
                        lhs = v2Tb[:, mc2 * 128: mc2 * 128 + npart]
                        selt = (sel8_sb if mc2 == 0 else sel2_sb)[0:npart, 0:ncls]
                        wtt = wt0 if mc2 == 0 else wt1
                        tps = psB.tile([128, 512], F32, tag="tprime")
                        nc.tensor.matmul(tps[0:npart, :], lhs,
                                         u2Tb[:, f0:f0 + 512],
                                         start=True, stop=True)
                        tpb = pBs.tile([128, 512], BF16, tag="tpb")
                        nc.scalar.copy(tpb[0:npart, :], tps[0:npart, :])
                        pm = pBs.tile([128, 512], BF16, tag="pm")
                        nc.vector.tensor_tensor(pm[0:npart, :],
                                                wtt[0:npart, f0:f0 + 512],
                                                tpb[0:npart, :], op=MUL)
                        pr = pBs.tile([128, 64], F32, tag="pr")
                        nc.vector.tensor_reduce(
                            pr[0:npart, :],
                            pm[0:npart, :].rearrange("p (r i) -> p r i", i=I),
                            axis=AXX, op=ADD)
                        mo = psM.tile([16, 64], F32, tag="mo")
                        nc.tensor.matmul(mo[0:ncls, :], selt, pr[0:npart, :],
                                         start=True, stop=True)
                        mtgt = mAll if mc2 == 0 else mAll2
                        nc.scalar.copy(mtgt[0:ncls, rb * 64:(rb + 1) * 64],
                                       mo[0:ncls, :])
                    if rb == 8:
                        nc.sync.dma_start(cc_in0.ap()[0:8], mAll[:, 0:RH])
                        nc.sync.dma_start(cc_in0.ap()[8:10], mAll2[:, 0:RH])
                        if getattr(nc, "_for_sim", False):
                            nc.sync.dma_start(cc_out0.ap(), cc_in0.ap())
                        else:
                            nc.gpsimd.collective_compute(
                                "AllReduce", ADD,
                                replica_groups=[list(range(NCORES))],
                                ins=[cc_in0.ap()], outs=[cc_out0.ap()])
                nc.sync.dma_start(cc_in1.ap()[0:8], mAll[:, RH:R])
                nc.sync.dma_start(cc_in1.ap()[8:10], mAll2[:, RH:R])
                if getattr(nc, "_for_sim", False):
                    nc.sync.dma_start(cc_out1.ap(), cc_in1.ap())
                else:
                    nc.gpsimd.collective_compute(
                        "AllReduce", ADD,
                        replica_groups=[list(range(NCORES))],
                        ins=[cc_in1.ap()], outs=[cc_out1.ap()])
                nc.sync.dma_start(msum[:, 0:RH], cc_out0.ap())
                nc.sync.dma_start(msum[:, RH:R], cc_out1.ap())
                bij = bijA if it == 0 else bijB
                nc.vector.tensor_scalar(bij[:], msum[:], 1.0 / B, None, op0=MUL)
                if it > 0:
                    nc.vector.tensor_add(bij[:], bij[:], bijA[:])
                # softmax over routes (free dim)
                rmax = pq.tile([C, 1], F32, tag="rmax")
                nc.vector.tensor_reduce(rmax[:], bij[:], axis=AXX, op=MAX)
                nrm = pq.tile([C, 1], F32, tag="nrm")
                nc.vector.tensor_scalar(nrm[:], rmax[:], -1.0, None, op0=MUL)
                nc.scalar.activation(csm[:], bij[:], ACT.Exp, bias=nrm[:])
                rsm = pq.tile([C, 1], F32, tag="rsm")
                nc.vector.tensor_reduce(rsm[:], csm[:], axis=AXX, op=ADD)
                rrc = pq.tile([C, 1], F32, tag="rrc")
                nc.vector.reciprocal(rrc[:], rsm[:])
                nc.vector.tensor_scalar(csm[:], csm[:], rrc[:], None, op0=MUL)


# ------------------------- host side ---------------------------------------
_CACHE = {}


def _pack(x, conv1_w, conv1_b, conv2_w, conv2_b, W):
    bf = ml_dtypes.bfloat16
    xf = np.asarray(x, np.float32).reshape(B, 784)
    w1 = np.ascontiguousarray(
        np.asarray(conv1_w, np.float32).reshape(256, 81).T).astype(bf)
    b1v = np.asarray(conv1_b, np.float32).reshape(2, 128).T.copy()
    w2 = np.asarray(conv2_w, np.float32).reshape(2, 128, 2, 128, 81)
    # [mc, co, kc, ci, khw] -> [mc, khw, kc, ci, co]
    w2 = np.ascontiguousarray(w2.transpose(0, 4, 2, 3, 1)).reshape(324, 128, 128).astype(bf)
    b2v = np.asarray(conv2_b, np.float32).reshape(2, 128).T.copy()
    Wf = np.asarray(W, np.float32)
    wl = np.ascontiguousarray(Wf.transpose(0, 3, 1, 2)).reshape(F, CO).astype(bf)
    wt = np.ascontiguousarray(Wf.transpose(1, 2, 0, 3)).reshape(CO, F).astype(bf)
    s8 = np.zeros((128, 8), np.float32)
    s8[np.arange(128), np.arange(128) // 16] = 1.0
    s2m = np.zeros((32, 2), np.float32)
    s2m[np.arange(32), np.arange(32) // 16] = 1.0
    s16 = np.zeros((128, 16), np.float32)
    s16[np.arange(128), np.arange(128) // 8] = 1.0
    srn = np.zeros((8, 128, 128), np.float32)
    for t1 in range(8):
        srn[t1, 16 * t1 + np.arange(128) // 8, np.arange(128)] = 1.0

    shared = {
        "w1t": w1, "b1": b1v, "w2s": w2, "b2": b2v, "wlb": wl, "wtf": wt,
        "sel8": s8, "sel2": s2m, "sel16": s16.astype(bf),
        "e16": np.ascontiguousarray(s16.T).astype(bf),
        "eye128": np.eye(128).astype(bf), "eyef": np.eye(16, dtype=np.float32),
        "selr": srn.astype(bf),
    }
    in_maps = []
    for c in range(NCORES):
        xc = xf[c * BL:(c + 1) * BL]                    # [64, 784]
        xT = np.ascontiguousarray(xc.T).astype(bf)      # [784, 64] b-inner
        xs = np.zeros((784 + 8) * BL, bf)
        xs[:784 * BL] = xT.reshape(-1)
        in_maps.append({"xin": xs, **shared})
    return in_maps


def kernel(x, conv1_w, conv1_b, conv2_w, conv2_b, W):
    if "nc" not in _CACHE:
        _CACHE["nc"] = build_nc()
    nc = _CACHE["nc"]
    in_maps = _pack(x, conv1_w, conv1_b, conv2_w, conv2_b, W)
    res = run_bass_kernel_spmd(nc, in_maps, list(range(NCORES)), trace=False)
    outs = [res.results[c]["out"] for c in range(NCORES)]
    return np.concatenate(outs, axis=0).reshape(B, C, O).astype(np.float32)


# revision 18
# speedup vs baseline: 2.1701x; 2.1701x over previous
"""CapsuleNet forward kernel for 8 Trainium2 NeuronCores.

Data-parallel over batch (64 images / core); the routing b_ij batch-mean
uses two half-sized AllReduces per iteration (second half overlaps the
first collective).  u_hat is never materialized.

v2: batch-innermost layout [feature, position, batch] end to end.
  conv1  : im2col A[81, (r,c,b)] built by 18 strided DMAs with 2.5KB
           segments; 100 N=512 matmuls with contiguous rhs; ReLU+bias
           fused into contiguous PSUM->SBUF crops on alternating engines
  conv2  : per mc half, 6 PSUM accumulators (one per output row oh);
           972 accumulating N=384 matmuls whose rhs is a 3-dim AP with
           contiguous 128B inner runs (full PE stream rate)
  capsule: conv2 output bounced through DRAM once ([co,s,b] write ->
           f-major [f=128t+p, b] read); squash factors per capsule
           computed f-major via selector matmuls (sum over i = partition
           groups of 8), expanded back through a 16->128 selector matmul;
           u2Tb (b-major) derived with 72 PE transposes
  routing: s_j^T = (c-scaled W)^T @ u2 72-tile accumulation; W*c built in
           ONE broadcast-AP multiply; agreement mean via rank-64 matmul +
           gpsimd mult + DVE group-reduce + selector matmuls; AllReduce
           in two r-halves so collective 0 overlaps compute of half 1.
"""

import numpy as np
import ml_dtypes

import concourse.bacc as bacc
import concourse.bass as bass
import concourse.mybir as mybir
import concourse.tile as tile
from concourse.bass_utils import run_bass_kernel_spmd

F32 = mybir.dt.float32
BF16 = mybir.dt.bfloat16
MUL = mybir.AluOpType.mult
ADD = mybir.AluOpType.add
MAX = mybir.AluOpType.max
AXX = mybir.AxisListType.X
ACT = mybir.ActivationFunctionType

NCORES = 8
B = 512
BL = B // NCORES        # 64 images per core
R, C, O, I = 1152, 10, 16, 8
F = R * I               # 9216
CO = C * O              # 160
KT = F // 128           # 72 f-tiles
S2 = 36                 # 6x6 conv2 positions per image
NIT = 3
RH = R // 2             # collective half


def _sub(ap, off, dims):
    """Arbitrary strided view (offset in elements, dims=[[step,count],..])."""
    return bass.AP(ap.tensor, ap.offset + off, [list(d) for d in dims])


def _pp(ap):
    """Partition pitch (elements per partition row) of an SBUF AP."""
    return ap.ap[0][0]


def build_nc(for_sim=False, reps=1):
    nc = bacc.Bacc("TRN2", target_bir_lowering=False, debug=False,
                   num_devices=1 if for_sim else NCORES)
    nc._for_sim = for_sim

    xin = nc.dram_tensor("xin", [(784 + 8) * BL], BF16, kind="ExternalInput").ap()
    w1t = nc.dram_tensor("w1t", [81, 256], BF16, kind="ExternalInput").ap()
    b1 = nc.dram_tensor("b1", [128, 2], F32, kind="ExternalInput").ap()
    w2s = nc.dram_tensor("w2s", [324, 128, 128], BF16, kind="ExternalInput").ap()
    b2 = nc.dram_tensor("b2", [128, 2], F32, kind="ExternalInput").ap()
    wlb = nc.dram_tensor("wlb", [F, CO], BF16, kind="ExternalInput").ap()
    wtf = nc.dram_tensor("wtf", [CO, F], BF16, kind="ExternalInput").ap()
    sel8 = nc.dram_tensor("sel8", [128, 8], F32, kind="ExternalInput").ap()
    sel2 = nc.dram_tensor("sel2", [32, 2], F32, kind="ExternalInput").ap()
    sel16 = nc.dram_tensor("sel16", [128, 16], BF16, kind="ExternalInput").ap()
    e16 = nc.dram_tensor("e16", [16, 128], BF16, kind="ExternalInput").ap()
    eye128 = nc.dram_tensor("eye128", [128, 128], BF16, kind="ExternalInput").ap()
    eyef = nc.dram_tensor("eyef", [16, 16], F32, kind="ExternalInput").ap()
    selr = nc.dram_tensor("selr", [8, 128, 128], BF16, kind="ExternalInput").ap()
    out = nc.dram_tensor("out", [BL, CO], F32, kind="ExternalOutput").ap()

    vd1 = nc.dram_tensor("vd1", [F, BL], BF16)          # f-major bounce
    cc_in0 = nc.dram_tensor("cc_in0", [C, RH], F32)
    cc_in1 = nc.dram_tensor("cc_in1", [C, RH], F32)
    cc_out0 = nc.dram_tensor("cc_out0", [C, RH], F32,
                             addr_space="Local" if for_sim else "Shared")
    cc_out1 = nc.dram_tensor("cc_out1", [C, RH], F32,
                             addr_space="Local" if for_sim else "Shared")

    with tile.TileContext(nc, num_cores=NCORES) as tc:
        for _rep in range(reps):
            _body(tc, nc, xin, w1t, b1, w2s, b2, wlb, wtf, sel8, sel2,
                  sel16, e16, eye128, eyef, selr, out, vd1,
                  cc_in0, cc_in1, cc_out0, cc_out1)
    nc.compile()
    return nc


def _body(tc, nc, xin, w1t, b1, w2s, b2, wlb, wtf, sel8, sel2, sel16, e16,
          eye128, eyef, selr, out, vd1, cc_in0, cc_in1, cc_out0, cc_out1):
    with tc.tile_pool(name="const", bufs=1) as pc, \
         tc.tile_pool(name="upers", bufs=1) as pU:

        w1t_sb = pc.tile([81, 256], BF16, tag="w1t")
        nc.sync.dma_start(w1t_sb[:], w1t)
        b1_sb = pc.tile([128, 2], F32, tag="b1")
        nc.sync.dma_start(b1_sb[:], b1)
        b2_sb = pc.tile([128, 2], F32, tag="b2")
        sel8_sb = pc.tile([128, 8], F32, tag="sel8")
        sel2_sb = pc.tile([32, 2], F32, tag="sel2")
        sel16_sb = pc.tile([128, 16], BF16, tag="sel16")
        e16_sb = pc.tile([16, 128], BF16, tag="e16")
        eye128_sb = pc.tile([128, 128], BF16, tag="eye128")
        eyef_sb = pc.tile([16, 16], F32, tag="eyef")
        selr_sb = pc.tile([128, 8 * 128], BF16, tag="selr")
        u2R = pU.tile([128, KT * BL], BF16, tag="u2R")   # squashed u, f-major

        # ============ Phase A/B: conv1 + conv2 + capsule formation =========
        with tc.tile_pool(name="pH", bufs=1) as pH, \
             tc.tile_pool(name="pW2", bufs=8) as pW2:

            h1 = [pH.tile([128, 20 * 20 * BL], BF16, tag=f"h1_{kc}",
                          name=f"h1_{kc}") for kc in range(2)]

            with tc.tile_pool(name="pA", bufs=1) as pA, \
                 tc.tile_pool(name="ps1", bufs=4, space="PSUM") as ps1:
                # im2col incl. 8 garbage cols per row: one fully contiguous
                # 71.7KB descriptor per tap.  A[kh*9+kw, ((r*28+c)*64+b)]
                # = xT[(r+kh)*28 + c+kw, b]; cols c>=20 are garbage (never
                # read by the matmuls below).
                A = pA.tile([81, 20 * 28 * BL], BF16, tag="A")
                pa = _pp(A[:])
                for kh in range(9):
                    dst = _sub(A[:], 9 * kh * pa,
                               [[pa, 9], [1, 20 * 28 * BL]])
                    src = _sub(xin, 28 * kh * BL,
                               [[BL, 9], [1, 20 * 28 * BL]])
                    nc.sync.dma_start(dst, src)

                # remaining consts after the latency-critical im2col loads
                nc.sync.dma_start(b2_sb[:], b2)
                nc.sync.dma_start(sel8_sb[:], sel8)
                nc.sync.dma_start(sel2_sb[:], sel2)
                nc.sync.dma_start(sel16_sb[:], sel16)
                nc.sync.dma_start(e16_sb[:], e16)
                nc.sync.dma_start(eye128_sb[:], eye128)
                nc.sync.dma_start(eyef_sb[:], eyef)
                nc.sync.dma_start(
                    _sub(selr_sb[:], 0,
                         [[_pp(selr_sb[:]), 128], [128, 8], [1, 128]]),
                    _sub(selr, 0, [[128, 128], [128 * 128, 8], [1, 128]]))

                flip = 0
                for mc in range(2):
                    lhsT = w1t_sb[:, mc * 128:(mc + 1) * 128]
                    for r in range(20):
                        for c0, nn in ((0, 512), (512, 512), (1024, 256)):
                            ps = ps1.tile([128, 512], F32, tag="c1ps")
                            nc.tensor.matmul(
                                ps[0:128, 0:nn], lhsT,
                                A[:, r * 28 * BL + c0:r * 28 * BL + c0 + nn],
                                start=True, stop=True)
                            dst = h1[mc][:, r * 20 * BL + c0:
                                         r * 20 * BL + c0 + nn]
                            bb = b1_sb[:, mc:mc + 1]
                            if flip % 2 == 0:
                                nc.vector.tensor_scalar(dst, ps[0:128, 0:nn],
                                                        bb, 0.0,
                                                        op0=ADD, op1=MAX)
                            else:
                                nc.scalar.activation(dst, ps[0:128, 0:nn],
                                                     ACT.Relu, bias=bb)
                            flip += 1

            # conv2 + f-major capsule squash, per mc half
            with tc.tile_pool(name="pV", bufs=2) as pV, \
                 tc.tile_pool(name="pRw", bufs=2) as pRw, \
                 tc.tile_pool(name="pSq", bufs=2) as pSq, \
                 tc.tile_pool(name="pGf", bufs=1) as pGf, \
                 tc.tile_pool(name="ps2", bufs=1, space="PSUM") as ps2, \
                 tc.tile_pool(name="psN", bufs=1, space="PSUM") as psN, \
                 tc.tile_pool(name="psE", bufs=1, space="PSUM") as psE:
                pph = _pp(h1[0][:])
                for mc in range(2):
                    pss = [ps2.tile([128, 6 * BL], F32, tag=f"c2ps{oh}",
                                    name=f"c2ps{oh}_{mc}") for oh in range(6)]
                    for kc in range(2):
                        for khw in range(81):
                            kh2, kw2 = khw // 9, khw % 9
                            wch = pW2.tile([128, 128], BF16, tag="wch")
                            nc.sync.dma_start(wch[:],
                                              w2s[mc * 162 + khw * 2 + kc])
                            lhsT = wch[:]
                            for oh in range(6):
                                rhs = _sub(h1[kc][:],
                                           ((2 * oh + kh2) * 20 + kw2) * BL,
                                           [[pph, 128], [2 * BL, 6], [1, BL]])
                                nc.tensor.matmul(
                                    pss[oh][:], lhsT, rhs,
                                    start=(kc == 0 and khw == 0),
                                    stop=(kc == 1 and khw == 80))
                    # bias add -> v[co, (s, b)] bf16
                    v = pV.tile([128, S2 * BL], BF16, tag="v")
                    for oh in range(6):
                        dst = v[:, oh * 6 * BL:(oh + 1) * 6 * BL]
                        if oh % 2 == 0:
                            nc.vector.tensor_scalar(dst, pss[oh][:],
                                                    b2_sb[:, mc:mc + 1],
                                                    None, op0=ADD)
                        else:
                            nc.scalar.add(dst, pss[oh][:],
                                          b2_sb[:, mc:mc + 1])
                    # bounce: vd1[f, b] with f = (mc*128+co)*36 + s, in two
                    # co-halves so write/read/square pipeline
                    for hf in range(2):
                        nc.sync.dma_start(
                            _sub(vd1.ap(), (mc * 128 + hf * 64) * S2 * BL,
                                 [[S2 * BL, 64], [1, S2 * BL]]),
                            v[hf * 64:(hf + 1) * 64, :])
                    # f-major read: tile t (of this mc half), partition p
                    # holds f = mc*4608 + 128*t + p; tiles 18t..18t+17 = co half
                    u2w = pRw.tile([128, 36 * BL], BF16, tag="u2w")
                    sq = pSq.tile([128, 36 * BL], BF16, tag="sq")
                    for hf in range(2):
                        nc.sync.dma_start(
                            _sub(u2w[:], hf * 18 * BL,
                                 [[_pp(u2w[:]), 128], [BL, 18], [1, BL]]),
                            _sub(vd1.ap(), (mc * 128 + hf * 64) * S2 * BL,
                                 [[BL, 128], [128 * BL, 18], [1, BL]]))
                        nc.vector.tensor_mul(
                            sq[:, hf * 18 * BL:(hf + 1) * 18 * BL],
                            u2w[:, hf * 18 * BL:(hf + 1) * 18 * BL],
                            u2w[:, hf * 18 * BL:(hf + 1) * 18 * BL])
                    for g in range(5):
                        t0, t1 = g * 8, min(g * 8 + 8, 36)
                        ncol = (t1 - t0) * BL
                        n2 = psN.tile([16, 512], F32, tag="n2")
                        for t in range(t0, t1):
                            nc.tensor.matmul(
                                n2[0:16, (t - t0) * BL:(t - t0 + 1) * BL],
                                sel16_sb[:], sq[:, t * BL:(t + 1) * BL],
                                start=True, stop=True)
                        # g = sq/((1+sq)sqrt(sq)) = sqrt(sq)/(1+sq)
                        srt = pGf.tile([16, 512], F32, tag="srt")
                        nc.scalar.sqrt(srt[0:16, 0:ncol], n2[0:16, 0:ncol])
                        den = pGf.tile([16, 512], F32, tag="den")
                        nc.vector.tensor_scalar(den[0:16, 0:ncol],
                                                n2[0:16, 0:ncol], 1.0, None,
                                                op0=ADD)
                        rc = pGf.tile([16, 512], F32, tag="rc")
                        nc.vector.reciprocal_approx_fast(rc[0:16, 0:ncol],
                                                         den[0:16, 0:ncol])
                        gf = pGf.tile([16, 512], BF16, tag="gf")
                        nc.gpsimd.tensor_tensor(gf[0:16, 0:ncol],
                                                srt[0:16, 0:ncol],
                                                rc[0:16, 0:ncol], op=MUL)
                        gx = psE.tile([128, 512], F32, tag="gx")
                        nc.tensor.matmul(gx[0:128, 0:ncol], e16_sb[:],
                                         gf[0:16, 0:ncol],
                                         start=True, stop=True)
                        nc.vector.tensor_tensor(
                            u2R[:, (mc * 36 + t0) * BL:(mc * 36 + t1) * BL],
                            u2w[:, t0 * BL:t1 * BL],
                            gx[0:128, 0:ncol], op=MUL)

        # ============ transposes + routing =================================
        with tc.tile_pool(name="pB", bufs=1) as pB, \
             tc.tile_pool(name="pBs", bufs=2) as pBs, \
             tc.tile_pool(name="psq2", bufs=1) as pq:

            u2Tb = pB.tile([BL, F], BF16, tag="u2Tb")    # squashed u, b-major
            wsb = pB.tile([128, KT * CO], BF16, tag="wsb")
            wsrc = _sub(wlb, 0, [[CO, 128], [128 * CO, KT], [1, CO]])
            wdst = _sub(wsb[:], 0, [[_pp(wsb[:]), 128], [CO, KT], [1, CO]])
            nc.sync.dma_start(wdst, wsrc)
            with tc.tile_pool(name="ptr", bufs=2, space="PSUM") as ptr:
                for t in range(KT):
                    pst = ptr.tile([BL, 128], BF16, tag="tr")
                    nc.tensor.transpose(pst[:], u2R[:, t * BL:(t + 1) * BL],
                                        eye128_sb[:])
                    if t % 2 == 0:
                        nc.vector.tensor_copy(u2Tb[:, t * 128:(t + 1) * 128],
                                              pst[:])
                    else:
                        nc.scalar.copy(u2Tb[:, t * 128:(t + 1) * 128], pst[:])

            _routing(tc, nc, pB, pBs, pq, u2R, u2Tb, wsb, wtf, out,
                     sel8_sb, sel2_sb, eyef_sb, selr_sb,
                     cc_in0, cc_in1, cc_out0, cc_out1)


def _routing(tc, nc, pB, pBs, pq, u2R, u2Tb, wsb, wtf, out,
             sel8_sb, sel2_sb, eyef_sb, selr_sb,
             cc_in0, cc_in1, cc_out0, cc_out1):
        with tc.tile_pool(name="psB", bufs=2, space="PSUM") as psB, \
             tc.tile_pool(name="psM", bufs=2, space="PSUM") as psM, \
             tc.tile_pool(name="psS", bufs=1, space="PSUM") as psS:

            wt0 = pB.tile([128, F], BF16, tag="wt0")
            nc.sync.dma_start(wt0[:], wtf[0:128])
            wt1 = pB.tile([32, F], BF16, tag="wt1")
            nc.sync.dma_start(wt1[:], wtf[128:160])
            wp = pB.tile([128, KT * CO], BF16, tag="wp")
            cE = pB.tile([128, KT * C], BF16, tag="cE")
            cTr = pB.tile([128, 9 * C], BF16, tag="cTr")
            mAll = pB.tile([8, R], F32, tag="mAll")
            mAll2 = pB.tile([2, R], F32, tag="mAll2")
            bijA = pB.tile([C, R], F32, tag="bijA")
            bijB = pB.tile([C, R], F32, tag="bijB")
            csm = pB.tile([C, R], F32, tag="csm")
            v2T = pB.tile([BL, CO], F32, tag="v2T")
            v2Tb = pB.tile([BL, CO], BF16, tag="v2Tb")
            msum = pB.tile([C, R], F32, tag="msum")

            lam = 1.0 / R
            for it in range(NIT):
                if it > 0:
                    # cTr[r%128, q*10+c] = csm[c, r]  (PE transpose, 9 blocks)
                    for q in range(9):
                        pst = psB.tile([128, C], F32, tag="ctr", name="ctr",
                                       bufs=1)
                        nc.tensor.transpose(pst[:],
                                            csm[:, q * 128:(q + 1) * 128],
                                            eyef_sb[0:C, 0:C])
                        nc.vector.tensor_copy(cTr[:, q * C:(q + 1) * C], pst[:])
                    # cE[p, t*10+c] = csm[c, 16t + p//8] via selector matmuls
                    for t1 in range(8):
                        pse = psB.tile([128, 9 * C], F32, tag="cexp",
                                       name="cexp", bufs=1)
                        nc.tensor.matmul(pse[:],
                                         selr_sb[:, t1 * 128:(t1 + 1) * 128],
                                         cTr[:], start=True, stop=True)
                        nc.vector.tensor_copy(
                            _sub(cE[:], t1 * C,
                                 [[_pp(cE[:]), 128], [8 * C, 9], [1, C]]),
                            pse[:])
                    # wp = wsb * cE broadcast over o: two halves in parallel
                    ppw = _pp(wp[:])
                    pps = _pp(wsb[:])
                    ppc = _pp(cE[:])
                    HT = 60
                    nc.vector.tensor_tensor(
                        _sub(wp[:], 0, [[ppw, 128], [CO, HT], [O, C], [1, O]]),
                        _sub(wsb[:], 0, [[pps, 128], [CO, HT], [O, C], [1, O]]),
                        _sub(cE[:], 0, [[ppc, 128], [C, HT], [1, C], [0, O]]),
                        op=MUL)
                    nc.gpsimd.tensor_tensor(
                        _sub(wp[:], HT * CO,
                             [[ppw, 128], [CO, KT - HT], [O, C], [1, O]]),
                        _sub(wsb[:], HT * CO,
                             [[pps, 128], [CO, KT - HT], [O, C], [1, O]]),
                        _sub(cE[:], HT * C,
                             [[ppc, 128], [C, KT - HT], [1, C], [0, O]]),
                        op=MUL)

                # s_j^T [b, co] over 72 accumulating K-tiles
                wcur = wsb if it == 0 else wp
                ssum = psS.tile([BL, CO], F32, tag="ssum")
                for t in range(KT):
                    nc.tensor.matmul(ssum[:], u2R[:, t * BL:(t + 1) * BL],
                                     wcur[:, t * CO:(t + 1) * CO],
                                     start=(t == 0), stop=(t == KT - 1))

                # v2 = squash(s) over o-groups of 16 (iter0 folds the 1/R scale)
                ssb = pq.tile([BL, CO], F32, tag="ssb")
                nc.vector.tensor_copy(ssb[:], ssum[:])
                svr = pq.tile([BL, CO], F32, tag="svr")
                nc.vector.tensor_mul(svr[:], ssb[:], ssb[:])
                sqv = pq.tile([BL, C], F32, tag="sqv")
                nc.vector.tensor_reduce(sqv[:],
                                        svr[:].rearrange("p (c o) -> p c o", o=O),
                                        axis=AXX, op=ADD)
                if it == 0:
                    nc.vector.tensor_scalar(sqv[:], sqv[:], lam * lam, None,
                                            op0=MUL)
                srtv = pq.tile([BL, C], F32, tag="srtv")
                nc.scalar.sqrt(srtv[:], sqv[:])
                dv2 = pq.tile([BL, C], F32, tag="dv2")
                nc.vector.scalar_tensor_tensor(dv2[:], sqv[:], 1.0, srtv[:],
                                               op0=ADD, op1=MUL)
                rcv = pq.tile([BL, C], F32, tag="rcv")
                nc.vector.reciprocal(rcv[:], dv2[:])
                gv = pq.tile([BL, C], F32, tag="gv")
                nc.vector.tensor_mul(gv[:], sqv[:], rcv[:])
                if it == 0:
                    nc.vector.tensor_scalar(gv[:], gv[:], lam, None, op0=MUL)
                # v2T[b, (c,o)] = ssb * gv broadcast over o: one pass
                ppv = _pp(v2T[:])
                pps2 = _pp(ssb[:])
                ppg = _pp(gv[:])
                nc.vector.tensor_tensor(
                    _sub(v2T[:], 0, [[ppv, BL], [O, C], [1, O]]),
                    _sub(ssb[:], 0, [[pps2, BL], [O, C], [1, O]]),
                    _sub(gv[:], 0, [[ppg, BL], [1, C], [0, O]]),
                    op=MUL)

                if it == NIT - 1:
                    nc.sync.dma_start(out, v2T[:])
                    break

                nc.vector.tensor_copy(v2Tb[:], v2T[:])
                # m[c, r] = sum_{o,i} Wt[(c,o),(r,i)] * (v2^T @ u2)[(c,o),(r,i)]
                # r-blocks of 64 outer so the collective can go out in halves
                for rb in range(18):
                    f0 = rb * 512
                    for mc2 in range(2):
                        npart = 128 if mc2 == 0 else 32
                        ncls = 8 if mc2 == 0 else 2
                        lhs = v2Tb[:, mc2 * 128: mc2 * 128 + npart]
                        selt = (sel8_sb if mc2 == 0 else sel2_sb)[0:npart, 0:ncls]
                        wtt = wt0 if mc2 == 0 else wt1
                        tps = psB.tile([128, 512], F32, tag="tprime")
                        nc.tensor.matmul(tps[0:npart, :], lhs,
                                         u2Tb[:, f0:f0 + 512],
                                         start=True, stop=True)
                        tpb = pBs.tile([128, 512], BF16, tag="tpb")
                        nc.scalar.copy(tpb[0:npart, :], tps[0:npart, :])
                        pm = pBs.tile([128, 512], BF16, tag="pm")
                        nc.vector.tensor_tensor(pm[0:npart, :],
                                                wtt[0:npart, f0:f0 + 512],
                                                tpb[0:npart, :], op=MUL)
                        pr = pBs.tile([128, 64], F32, tag="pr")
                        nc.vector.tensor_reduce(
                            pr[0:npart, :],
                            pm[0:npart, :].rearrange("p (r i) -> p r i", i=I),
                            axis=AXX, op=ADD)
                        mo = psM.tile([16, 64], F32, tag="mo")
                        nc.tensor.matmul(mo[0:ncls, :], selt, pr[0:npart, :],
                                         start=True, stop=True)
                        mtgt = mAll if mc2 == 0 else mAll2
                        nc.scalar.copy(mtgt[0:ncls, rb * 64:(rb + 1) * 64],
                                       mo[0:ncls, :])
                    if rb == 8:
                        nc.sync.dma_start(cc_in0.ap()[0:8], mAll[:, 0:RH])
                        nc.sync.dma_start(cc_in0.ap()[8:10], mAll2[:, 0:RH])
                        if getattr(nc, "_for_sim", False):
                            nc.sync.dma_start(cc_out0.ap(), cc_in0.ap())
                        else:
                            nc.gpsimd.collective_compute(
                                "AllReduce", ADD,
                                replica_groups=[list(range(NCORES))],
                                ins=[cc_in0.ap()], outs=[cc_out0.ap()])
                nc.sync.dma_start(cc_in1.ap()[0:8], mAll[:, RH:R])
                nc.sync.dma_start(cc_in1.ap()[8:10], mAll2[:, RH:R])
                if getattr(nc, "_for_sim", False):
                    nc.sync.dma_start(cc_out1.ap(), cc_in1.ap())
                else:
                    nc.gpsimd.collective_compute(
                        "AllReduce", ADD,
                        replica_groups=[list(range(NCORES))],
                        ins=[cc_in1.ap()], outs=[cc_out1.ap()])
                nc.sync.dma_start(msum[:, 0:RH], cc_out0.ap())
                nc.sync.dma_start(msum[:, RH:R], cc_out1.ap())
                bij = bijA if it == 0 else bijB
                nc.vector.tensor_scalar(bij[:], msum[:], 1.0 / B, None, op0=MUL)
                if it > 0:
                    nc.vector.tensor_add(bij[:], bij[:], bijA[:])
                # softmax over routes (free dim)
                rmax = pq.tile([C, 1], F32, tag="rmax")
                nc.vector.tensor_reduce(rmax[:], bij[:], axis=AXX, op=MAX)
                nrm = pq.tile([C, 1], F32, tag="nrm")
                nc.vector.tensor_scalar(nrm[:], rmax[:], -1.0, None, op0=MUL)
                nc.scalar.activation(csm[:], bij[:], ACT.Exp, bias=nrm[:])
                rsm = pq.tile([C, 1], F32, tag="rsm")
                nc.vector.tensor_reduce(rsm[:], csm[:], axis=AXX, op=ADD)
                rrc = pq.tile([C, 1], F32, tag="rrc")
                nc.vector.reciprocal(rrc[:], rsm[:])
                nc.vector.tensor_scalar(csm[:], csm[:], rrc[:], None, op0=MUL)


# ------------------------- host side ---------------------------------------
_CACHE = {}


def _pack(x, conv1_w, conv1_b, conv2_w, conv2_b, W):
    bf = ml_dtypes.bfloat16
    xf = np.asarray(x, np.float32).reshape(B, 784)
    w1 = np.ascontiguousarray(
        np.asarray(conv1_w, np.float32).reshape(256, 81).T).astype(bf)
    b1v = np.asarray(conv1_b, np.float32).reshape(2, 128).T.copy()
    w2 = np.asarray(conv2_w, np.float32).reshape(2, 128, 2, 128, 81)
    # [mc, co, kc, ci, khw] -> [mc, khw, kc, ci, co]
    w2 = np.ascontiguousarray(w2.transpose(0, 4, 2, 3, 1)).reshape(324, 128, 128).astype(bf)
    b2v = np.asarray(conv2_b, np.float32).reshape(2, 128).T.copy()
    Wf = np.asarray(W, np.float32)
    wl = np.ascontiguousarray(Wf.transpose(0, 3, 1, 2)).reshape(F, CO).astype(bf)
    wt = np.ascontiguousarray(Wf.transpose(1, 2, 0, 3)).reshape(CO, F).astype(bf)
    s8 = np.zeros((128, 8), np.float32)
    s8[np.arange(128), np.arange(128) // 16] = 1.0
    s2m = np.zeros((32, 2), np.float32)
    s2m[np.arange(32), np.arange(32) // 16] = 1.0
    s16 = np.zeros((128, 16), np.float32)
    s16[np.arange(128), np.arange(128) // 8] = 1.0
    srn = np.zeros((8, 128, 128), np.float32)
    for t1 in range(8):
        srn[t1, 16 * t1 + np.arange(128) // 8, np.arange(128)] = 1.0

    shared = {
        "w1t": w1, "b1": b1v, "w2s": w2, "b2": b2v, "wlb": wl, "wtf": wt,
        "sel8": s8, "sel2": s2m, "sel16": s16.astype(bf),
        "e16": np.ascontiguousarray(s16.T).astype(bf),
        "eye128": np.eye(128).astype(bf), "eyef": np.eye(16, dtype=np.float32),
        "selr": srn.astype(bf),
    }
    in_maps = []
    for c in range(NCORES):
        xc = xf[c * BL:(c + 1) * BL]                    # [64, 784]
        xT = np.ascontiguousarray(xc.T).astype(bf)      # [784, 64] b-inner
        xs = np.zeros((784 + 8) * BL, bf)
        xs[:784 * BL] = xT.reshape(-1)
        in_maps.append({"xin": xs, **shared})
    return in_maps


def kernel(x, conv1_w, conv1_b, conv2_w, conv2_b, W):
    if "nc" not in _CACHE:
        _CACHE["nc"] = build_nc()
    nc = _CACHE["nc"]
    in_maps = _pack(x, conv1_w, conv1_b, conv2_w, conv2_b, W)
    res = run_bass_kernel_spmd(nc, in_maps, list(range(NCORES)), trace=False)
    outs = [res.results[c]["out"] for c in range(NCORES)]
    return np.concatenate(outs, axis=0).reshape(B, C, O).astype(np.float32)
